# revision 29
# baseline (speedup 1.0000x reference)
"""Dcls1d (dilated conv1d with learnable spacings) on 8 Trainium2 NeuronCores.

Problem: x (8, 256, 2048) f32; weight (256, 256, 16); P (1, 256, 256, 16);
bias (256,). A dense conv kernel (O=256, I=256, DKS=33) is built from
weight/P by linear interpolation at positions P, then conv1d(x, kern,
pad=16) + bias -> out (8, 256, 2048).

Strategy:
 - Host: fold (weight, P) -> per-tap matmul weights, keeping only taps that
   are actually nonzero (P = clip(0.5*randn, +-16) clusters around the
   center tap, so typically only ~7 of 33 taps carry weight). Edge taps are
   extremely sparse (a handful of nonzero input channels) - those become
   32-row "strip" matmuls packed into PE row-tiles instead of full 128-deep
   matmuls.
 - Device: data-parallel over batch, one batch element per NeuronCore.
   out[o, t] = sum_d kern[d][:, o] . x[:, t + d - 16], accumulated in PSUM
   over 2 input-channel chunks x dense taps (+ sparse strips), 2x4 output
   tiles of (128, 512); bias added on the PSUM->SBUF move.
"""

import numpy as np

try:
    import concourse  # noqa: F401
except ImportError:  # pragma: no cover - container fallback
    import sys

    sys.path.insert(0, "/opt/trn_rl_repo")

import concourse.bacc as bacc
import concourse.mybir as mybir
import concourse.tile as tile
import concourse.bass_utils as bass_utils

DKS = 33
PAD = 16
N, IC, LEN = 8, 256, 2048
OC = 256
KC = 16
N_CORES = 8
SPARSE_ROWS = 128  # packed contraction depth: full 128 partitions
# (64-partition operands stream at half SBUF bandwidth on the PE)

TRACE = False  # test harness sets kernel_mod.TRACE = True to profile
DTYPE = "f32r"  # "f32r" (safe, ~1.5e-4 rel err) or "bf16" (faster, ~3e-3)
LAST_EXEC_NS = None
LAST_TRACE_PATH = None

_BUILD_CACHE = {}


def _host_fold_kernel(weight, P):
    """Reproduce reference construct_kernel for the active taps only.

    Returns (dmin, ktaps) with ktaps[t, i, o] the lhsT-layout weights for
    tap d = dmin + t. Mirrors the reference arithmetic in fp32:
    kern[o,i,d] = sum_kc w[o,i,kc] * (W1 + frac*(W2-W1)).
    """
    w = np.asarray(weight, dtype=np.float32)
    Pf32 = np.asarray(P, dtype=np.float32)
    Pp = Pf32 + np.float32(DKS // 2)
    Pf = np.floor(Pp)
    frac = (Pp - Pf)[0, 0]  # (IC, KC) - out-channel 0's fractional part
    P1 = Pf[0]  # (OC, IC, KC)

    dmin = max(0, int(P1.min()))
    dmax = min(DKS - 1, int(P1.max()) + 1)
    dd = np.arange(dmin, dmax + 1, dtype=np.float32)
    W1 = dd[:, None, None, None] == P1[None]
    W2 = dd[:, None, None, None] == (P1 + 1)[None]
    K = W1.astype(np.float32) + frac[None, None] * (
        W2.astype(np.float32) - W1.astype(np.float32)
    )
    kern = (w[None] * K).sum(-1)  # (T, OC, IC)
    ktaps = np.ascontiguousarray(kern.transpose(0, 2, 1))  # (T, IC, OC)
    return dmin, ktaps


def _classify_taps(ktaps):
    """Split taps into dense ones and sparse (tap, nonzero-rows) ones.

    Sparse taps' (tap, input-row) pairs are packed -- with the row's x data
    pre-shifted by the tap offset on the host -- into one K<=SPARSE_ROWS
    matmul, so the tap budget is a row budget.
    """
    T = ktaps.shape[0]
    nzrows = [np.nonzero(np.any(ktaps[t] != 0, axis=1))[0] for t in range(T)]
    order = sorted(range(T), key=lambda t: len(nzrows[t]))
    strips = []
    sparse = set()
    used = 0
    for t in order:
        n = len(nzrows[t])
        if n == 0:
            sparse.add(t)  # all-zero tap: drop entirely
        elif n <= 48 and used + n <= SPARSE_ROWS:
            strips.append((t, nzrows[t]))
            sparse.add(t)
            used += n
    dense = [t for t in range(T) if t not in sparse]
    strips.sort()
    return dense, strips


def _build(T, dense, strip_taps, dtype_name):
    f32 = mybir.dt.float32
    pdt = mybir.dt.float32r if dtype_name == "f32r" else mybir.dt.bfloat16

    W = LEN + T - 1  # host-padded x width; tap t reads cols [t+c0, t+c0+512)
    n_tc = LEN // 512
    Td = len(dense)
    S = len(strip_taps)

    nc = bacc.Bacc("TRN2", target_bir_lowering=False, debug=False,
                   num_devices=N_CORES)
    x_d = nc.dram_tensor("x", (2, 128, W), pdt, kind="ExternalInput")
    kt_d = nc.dram_tensor("kt", (2, 128, Td, OC), pdt, kind="ExternalInput")
    b_d = nc.dram_tensor("bias", (128, 2), f32, kind="ExternalInput")
    if S:
        ktp_d = nc.dram_tensor("ktp", (SPARSE_ROWS, OC), pdt,
                               kind="ExternalInput")
        xg_d = nc.dram_tensor("xg", (SPARSE_ROWS, LEN), pdt,
                              kind="ExternalInput")
    y_d = nc.dram_tensor("out", (2, 128, LEN), f32, kind="ExternalOutput")

    with tile.TileContext(nc) as tc:
        with (
            tc.tile_pool(name="const", bufs=1) as cpool,
            tc.tile_pool(name="ps", bufs=8, space="PSUM") as pspool,
            tc.tile_pool(name="outp", bufs=4) as opool,
        ):
            kt_t = [cpool.tile([128, Td, OC], pdt, tag=f"kt{ic}",
                               name=f"kt{ic}") for ic in range(2)]
            xp = [cpool.tile([128, W], pdt, tag=f"xp{ic}",
                             name=f"xp{ic}") for ic in range(2)]
            bias_t = cpool.tile([128, 2], f32, tag="bias")
            if S:
                ktp_t = cpool.tile([SPARSE_ROWS, OC], pdt, tag="ktp",
                                   name="ktp")
                xg_t = cpool.tile([SPARSE_ROWS, LEN], pdt, tag="xg",
                                  name="xg")

            # PE warmup: the HAM clock gate holds the PE at 1.2 GHz until it
            # has been busy ~3.4us; dummy matmuls cover the DMA fill so real
            # matmuls run at 2.4 GHz.
            warm = cpool.tile([128, 64], pdt, tag="warm")
            nc.vector.memset(
                warm[:].bitcast(f32 if dtype_name == "f32r"
                                else mybir.dt.bfloat16), 0.0)
            wps = pspool.tile([64, 64], f32, tag="ps", name="warm_ps")
            for i in range(50):
                nc.tensor.matmul(wps[:], warm[:, 0:64], warm[:],
                                 start=True, stop=True)

            # DMA completion is ~serial per stream (sync / scalar each drive
            # their own HWDGE ring); order each stream by when the PE needs
            # the data. The matmul stream is phase-split: all 8 output
            # groups' ic0 halves first (needing only kt0 + xp0), then ic1.
            dh = max(1, min(3, Td))  # first kt chunk: dense taps [0, dh)
            ch = 1024 + T - 1  # xp column split: tcn0+1 read [0, ch)
            cq = min(768, ch)  # tcn0 window only needs cols [0, 512+T)
            nc.sync.dma_start(kt_t[0][:, 0:dh], kt_d.ap()[0][:, 0:dh])
            nc.scalar.dma_start(xp[0][:, 0:cq], x_d.ap()[0][:, 0:cq])
            if dh < Td:
                nc.sync.dma_start(kt_t[0][:, dh:Td], kt_d.ap()[0][:, dh:Td])
            nc.scalar.dma_start(xp[0][:, cq:ch], x_d.ap()[0][:, cq:ch])
            nc.scalar.dma_start(xp[0][:, ch:W], x_d.ap()[0][:, ch:W])
            nc.sync.dma_start(kt_t[1][:, 0:dh], kt_d.ap()[1][:, 0:dh])
            nc.scalar.dma_start(xp[1][:, 0:cq], x_d.ap()[1][:, 0:cq])
            if dh < Td:
                nc.sync.dma_start(kt_t[1][:, dh:Td], kt_d.ap()[1][:, dh:Td])
            nc.scalar.dma_start(xp[1][:, cq:W], x_d.ap()[1][:, cq:W])
            nc.sync.dma_start(bias_t[:], b_d.ap())
            if S:
                nc.sync.dma_start(ktp_t[:], ktp_d.ap())
                nc.scalar.dma_start(xg_t[:], xg_d.ap())

            ps = {}
            for tcn in range(n_tc):
                for oc in range(2):
                    ps[tcn, oc] = pspool.tile([128, 512], f32, tag="ps",
                                              name=f"ps_{tcn}_{oc}")

            def mm(ic, tcn, oc, td):
                nc.tensor.matmul(
                    ps[tcn, oc][:],
                    kt_t[ic][:, td, oc * 128:(oc + 1) * 128],
                    xp[ic][:, dense[td] + tcn * 512:
                           dense[td] + tcn * 512 + 512],
                    start=(ic == 0 and td == 0),
                    stop=False,
                )

            def strip_mms(tcn, oc):
                # all sparse taps in one K=SPARSE_ROWS matmul: each packed
                # row's x was pre-shifted by its tap offset on the host, so
                # every row reads the same column window
                c0 = tcn * 512
                nc.tensor.matmul(
                    ps[tcn, oc][:],
                    ktp_t[:, oc * 128:(oc + 1) * 128],
                    xg_t[:, c0:c0 + 512],
                    start=False, stop=True,
                )

            def close(tcn, oc):
                if S:
                    strip_mms(tcn, oc)
                else:
                    # re-emit nothing; mark group end via last dense mm's
                    # stop flag instead
                    pass
                ot = opool.tile([128, 512], f32, tag="out",
                                name=f"ot_{tcn}_{oc}")
                last = (tcn == n_tc - 1 and oc == 1)
                if not last:
                    nc.vector.tensor_scalar(
                        ot[:], ps[tcn, oc][:], bias_t[:, oc:oc + 1],
                        None, mybir.AluOpType.add,
                    )
                    nc.sync.dma_start(
                        y_d.ap()[oc][:, tcn * 512:(tcn + 1) * 512], ot[:],
                    )
                else:
                    # split the final store across engines to trim the tail
                    nc.vector.tensor_scalar(
                        ot[:, 0:256], ps[tcn, oc][:, 0:256],
                        bias_t[:, oc:oc + 1], None, mybir.AluOpType.add,
                    )
                    nc.scalar.activation(
                        ot[:, 256:512], ps[tcn, oc][:, 256:512],
                        mybir.ActivationFunctionType.Identity,
                        bias=bias_t[:, oc:oc + 1],
                    )
                    c0 = tcn * 512
                    nc.sync.dma_start(
                        y_d.ap()[oc][:, c0:c0 + 256], ot[:, 0:256])
                    nc.scalar.dma_start(
                        y_d.ap()[oc][:, c0 + 256:c0 + 512], ot[:, 256:512])

            for ic in range(2):
                for tcn in range(n_tc):
                    for oc in range(2):
                        for td in range(Td):
                            if ic == 1 and td == Td - 1 and not S:
                                nc.tensor.matmul(
                                    ps[tcn, oc][:],
                                    kt_t[ic][:, td,
                                             oc * 128:(oc + 1) * 128],
                                    xp[ic][:, dense[td] + tcn * 512:
                                           dense[td] + tcn * 512 + 512],
                                    start=False, stop=True,
                                )
                            else:
                                mm(ic, tcn, oc, td)
                        if ic == 1:
                            close(tcn, oc)

    nc.compile()
    return nc


def kernel(x, weight, P, bias):
    global LAST_EXEC_NS, LAST_TRACE_PATH
    x = np.ascontiguousarray(np.asarray(x, dtype=np.float32))
    bias = np.asarray(bias, dtype=np.float32)

    dmin, ktaps = _host_fold_kernel(weight, P)
    T = ktaps.shape[0]
    dense, strips = _classify_taps(ktaps)
    if not dense:  # degenerate: keep at least one dense tap
        if strips:
            dense = [strips[-1][0]]
            strips = [s for s in strips if s[0] != dense[0]]
        else:
            dense = [0]  # all-zero kernel: output is just the bias
    strip_taps = tuple(t for t, _ in strips)
    S = len(strips)

    if DTYPE == "bf16":
        import ml_dtypes
        pdt_np = ml_dtypes.bfloat16
    else:
        pdt_np = np.float32

    key = (T, tuple(dense), strip_taps, DTYPE)
    if key not in _BUILD_CACHE:
        _BUILD_CACHE[key] = _build(T, dense, strip_taps, DTYPE)
    nc = _BUILD_CACHE[key]

    # kt layout (ic_chunk, i_within, dense_tap, o): per-ic rows contiguous
    kt_in = np.ascontiguousarray(
        ktaps[dense].reshape(len(dense), 2, 128, OC)
        .transpose(1, 2, 0, 3).astype(pdt_np))
    b_in = np.ascontiguousarray(bias.reshape(2, 128).T)

    # pad x on the host so the device needs no memset: column c of xpad is
    # x index c + (dmin - PAD)
    W = LEN + T - 1
    zl = max(0, PAD - dmin)
    xs = max(0, dmin - PAD)
    xn = min(LEN - xs, W - zl)
    xpad = np.zeros((N_CORES, 2, 128, W), dtype=pdt_np)
    xpad[:, :, :, zl:zl + xn] = (
        x.reshape(N_CORES, 2, 128, LEN)[:, :, :, xs:xs + xn].astype(pdt_np))

    if S:
        ktp_in = np.zeros((SPARSE_ROWS, OC), dtype=pdt_np)
        xg_in = np.zeros((N_CORES, SPARSE_ROWS, LEN), dtype=pdt_np)
        flat_x = xpad.reshape(N_CORES, 256, W)
        r0 = 0
        for t_sp, rows in strips:
            nr = len(rows)
            ktp_in[r0:r0 + nr] = ktaps[t_sp][rows].astype(pdt_np)
            # pre-shift: packed row reads xpad[row, c + t_sp]
            xg_in[:, r0:r0 + nr] = flat_x[:, rows, t_sp:t_sp + LEN]
            r0 += nr

    in_maps = []
    for c in range(N_CORES):
        m = {"x": xpad[c], "kt": kt_in, "bias": b_in}
        if S:
            m["ktp"] = ktp_in
            m["xg"] = xg_in[c]
        in_maps.append(m)

    kwargs = {}
    bass_utils.upload_artifacts = lambda tmpdir: tmpdir
    if TRACE:
        kwargs["trace"] = True
    res = None
    for attempt in range(3):
        try:
            res = bass_utils.run_bass_kernel_spmd(
                nc, in_maps, core_ids=list(range(N_CORES)), **kwargs
            )
            break
        except Exception:
            # occasional transient NRT_EXEC_UNIT_UNRECOVERABLE on this
            # fabric; retry once or twice before giving up
            if attempt == 2:
                raise
    if TRACE:
        LAST_EXEC_NS = res.exec_time_ns
        if res.instructions_and_trace is not None:
            LAST_TRACE_PATH = res.instructions_and_trace[1]

    out = np.empty((N, OC, LEN), dtype=np.float32)
    for c in range(N_CORES):
        out[c] = res.results[c]["out"].reshape(OC, LEN)
    return out


# revision 30
# speedup vs baseline: 1.0335x; 1.0335x over previous
"""Dcls1d (dilated conv1d with learnable spacings) on 8 Trainium2 NeuronCores.

Problem: x (8, 256, 2048) f32; weight (256, 256, 16); P (1, 256, 256, 16);
bias (256,). A dense conv kernel (O=256, I=256, DKS=33) is built from
weight/P by linear interpolation at positions P, then conv1d(x, kern,
pad=16) + bias -> out (8, 256, 2048).

Strategy:
 - Host: fold (weight, P) -> per-tap matmul weights, keeping only taps that
   are actually nonzero (P = clip(0.5*randn, +-16) clusters around the
   center tap, so typically only ~7 of 33 taps carry weight). Edge taps are
   extremely sparse (a handful of nonzero input channels) - those become
   32-row "strip" matmuls packed into PE row-tiles instead of full 128-deep
   matmuls.
 - Device: data-parallel over batch, one batch element per NeuronCore.
   out[o, t] = sum_d kern[d][:, o] . x[:, t + d - 16], accumulated in PSUM
   over 2 input-channel chunks x dense taps (+ sparse strips), 2x4 output
   tiles of (128, 512); bias added on the PSUM->SBUF move.
"""

import numpy as np

try:
    import concourse  # noqa: F401
except ImportError:  # pragma: no cover - container fallback
    import sys

    sys.path.insert(0, "/opt/trn_rl_repo")

import concourse.bacc as bacc
import concourse.mybir as mybir
import concourse.tile as tile
import concourse.bass_utils as bass_utils

DKS = 33
PAD = 16
N, IC, LEN = 8, 256, 2048
OC = 256
KC = 16
N_CORES = 8
SPARSE_ROWS = 128  # packed contraction depth: full 128 partitions
# (64-partition operands stream at half SBUF bandwidth on the PE)

TRACE = False  # test harness sets kernel_mod.TRACE = True to profile
DTYPE = "f32r"  # "f32r" (safe, ~1.5e-4 rel err) or "bf16" (faster, ~3e-3)
LAST_EXEC_NS = None
LAST_TRACE_PATH = None

_BUILD_CACHE = {}


def _host_fold_kernel(weight, P):
    """Reproduce reference construct_kernel for the active taps only.

    Returns (dmin, ktaps) with ktaps[t, i, o] the lhsT-layout weights for
    tap d = dmin + t. Mirrors the reference arithmetic in fp32:
    kern[o,i,d] = sum_kc w[o,i,kc] * (W1 + frac*(W2-W1)).
    """
    w = np.asarray(weight, dtype=np.float32)
    Pf32 = np.asarray(P, dtype=np.float32)
    Pp = Pf32 + np.float32(DKS // 2)
    Pf = np.floor(Pp)
    frac = (Pp - Pf)[0, 0]  # (IC, KC) - out-channel 0's fractional part
    P1 = Pf[0]  # (OC, IC, KC)

    dmin = max(0, int(P1.min()))
    dmax = min(DKS - 1, int(P1.max()) + 1)
    dd = np.arange(dmin, dmax + 1, dtype=np.float32)
    W1 = dd[:, None, None, None] == P1[None]
    W2 = dd[:, None, None, None] == (P1 + 1)[None]
    K = W1.astype(np.float32) + frac[None, None] * (
        W2.astype(np.float32) - W1.astype(np.float32)
    )
    kern = (w[None] * K).sum(-1)  # (T, OC, IC)
    ktaps = np.ascontiguousarray(kern.transpose(0, 2, 1))  # (T, IC, OC)
    return dmin, ktaps


def _classify_taps(ktaps):
    """Split taps into dense ones and sparse (tap, nonzero-rows) ones.

    Sparse taps' (tap, input-row) pairs are packed -- with the row's x data
    pre-shifted by the tap offset on the host -- into one K<=SPARSE_ROWS
    matmul, so the tap budget is a row budget.
    """
    T = ktaps.shape[0]
    nzrows = [np.nonzero(np.any(ktaps[t] != 0, axis=1))[0] for t in range(T)]
    order = sorted(range(T), key=lambda t: len(nzrows[t]))
    strips = []
    sparse = set()
    used = 0
    for t in order:
        n = len(nzrows[t])
        if n == 0:
            sparse.add(t)  # all-zero tap: drop entirely
        elif n <= 48 and used + n <= SPARSE_ROWS:
            strips.append((t, nzrows[t]))
            sparse.add(t)
            used += n
    dense = [t for t in range(T) if t not in sparse]
    strips.sort()
    return dense, strips


def _build(T, dense, strip_taps, dtype_name):
    f32 = mybir.dt.float32
    pdt = mybir.dt.float32r if dtype_name == "f32r" else mybir.dt.bfloat16

    W = LEN + T - 1  # host-padded x width; tap t reads cols [t+c0, t+c0+512)
    n_tc = LEN // 512
    Td = len(dense)
    S = len(strip_taps)

    nc = bacc.Bacc("TRN2", target_bir_lowering=False, debug=False,
                   num_devices=N_CORES)
    x_d = nc.dram_tensor("x", (2, 128, W), pdt, kind="ExternalInput")
    kt_d = nc.dram_tensor("kt", (2, 128, Td, OC), pdt, kind="ExternalInput")
    b_d = nc.dram_tensor("bias", (128, 2), f32, kind="ExternalInput")
    if S:
        ktp_d = nc.dram_tensor("ktp", (SPARSE_ROWS, OC), pdt,
                               kind="ExternalInput")
        xg_d = nc.dram_tensor("xg", (SPARSE_ROWS, LEN), pdt,
                              kind="ExternalInput")
    y_d = nc.dram_tensor("out", (2, 128, LEN), f32, kind="ExternalOutput")

    with tile.TileContext(nc) as tc:
        with (
            tc.tile_pool(name="const", bufs=1) as cpool,
            tc.tile_pool(name="ps", bufs=8, space="PSUM") as pspool,
            tc.tile_pool(name="outp", bufs=4) as opool,
        ):
            kt_t = [cpool.tile([128, Td, OC], pdt, tag=f"kt{ic}",
                               name=f"kt{ic}") for ic in range(2)]
            xp = [cpool.tile([128, W], pdt, tag=f"xp{ic}",
                             name=f"xp{ic}") for ic in range(2)]
            bias_t = cpool.tile([128, 2], f32, tag="bias")
            if S:
                ktp_t = cpool.tile([SPARSE_ROWS, OC], pdt, tag="ktp",
                                   name="ktp")
                xg_t = cpool.tile([SPARSE_ROWS, LEN], pdt, tag="xg",
                                  name="xg")

            # PE warmup: the HAM clock gate holds the PE at 1.2 GHz until it
            # has been busy ~3.4us; dummy matmuls cover the DMA fill so real
            # matmuls run at 2.4 GHz.
            warm = cpool.tile([128, 64], pdt, tag="warm")
            nc.vector.memset(
                warm[:].bitcast(f32 if dtype_name == "f32r"
                                else mybir.dt.bfloat16), 0.0)
            wps = pspool.tile([64, 64], f32, tag="ps", name="warm_ps")
            for i in range(50):
                nc.tensor.matmul(wps[:], warm[:, 0:64], warm[:],
                                 start=True, stop=True)

            # DMA completion is ~serial per stream (sync / scalar each drive
            # their own HWDGE ring); order each stream by when the PE needs
            # the data. The matmul stream is phase-split: all 8 output
            # groups' ic0 halves first (needing only kt0 + xp0), then ic1.
            dh = max(1, min(3, Td))  # first kt chunk: dense taps [0, dh)
            ch = 1024 + T - 1  # xp column split: tcn0+1 read [0, ch)
            cq = min(768, ch)  # tcn0 window only needs cols [0, 512+T)
            nc.sync.dma_start(kt_t[0][:, 0:dh], kt_d.ap()[0][:, 0:dh])
            nc.scalar.dma_start(xp[0][:, 0:cq], x_d.ap()[0][:, 0:cq])
            if dh < Td:
                nc.sync.dma_start(kt_t[0][:, dh:Td], kt_d.ap()[0][:, dh:Td])
            nc.scalar.dma_start(xp[0][:, cq:ch], x_d.ap()[0][:, cq:ch])
            nc.scalar.dma_start(xp[0][:, ch:W], x_d.ap()[0][:, ch:W])
            nc.sync.dma_start(kt_t[1][:, 0:dh], kt_d.ap()[1][:, 0:dh])
            nc.scalar.dma_start(xp[1][:, 0:cq], x_d.ap()[1][:, 0:cq])
            if dh < Td:
                nc.sync.dma_start(kt_t[1][:, dh:Td], kt_d.ap()[1][:, dh:Td])
            nc.scalar.dma_start(xp[1][:, cq:W], x_d.ap()[1][:, cq:W])
            nc.sync.dma_start(bias_t[:], b_d.ap())
            if S:
                nc.sync.dma_start(ktp_t[:], ktp_d.ap())
                nc.scalar.dma_start(xg_t[:], xg_d.ap())

            ps = {}
            for tcn in range(n_tc):
                for oc in range(2):
                    ps[tcn, oc] = pspool.tile([128, 512], f32, tag="ps",
                                              name=f"ps_{tcn}_{oc}")

            def mm(ic, tcn, oc, td):
                nc.tensor.matmul(
                    ps[tcn, oc][:],
                    kt_t[ic][:, td, oc * 128:(oc + 1) * 128],
                    xp[ic][:, dense[td] + tcn * 512:
                           dense[td] + tcn * 512 + 512],
                    start=(ic == 0 and td == 0),
                    stop=False,
                )

            def strip_mms(tcn, oc):
                # all sparse taps in one K=SPARSE_ROWS matmul: each packed
                # row's x was pre-shifted by its tap offset on the host, so
                # every row reads the same column window
                c0 = tcn * 512
                nc.tensor.matmul(
                    ps[tcn, oc][:],
                    ktp_t[:, oc * 128:(oc + 1) * 128],
                    xg_t[:, c0:c0 + 512],
                    start=False, stop=True,
                )

            def close(tcn, oc):
                if S:
                    strip_mms(tcn, oc)
                else:
                    # re-emit nothing; mark group end via last dense mm's
                    # stop flag instead
                    pass
                ot = opool.tile([128, 512], f32, tag="out",
                                name=f"ot_{tcn}_{oc}")
                last = (tcn == n_tc - 1 and oc == 1)
                if not last:
                    nc.vector.tensor_scalar(
                        ot[:], ps[tcn, oc][:], bias_t[:, oc:oc + 1],
                        None, mybir.AluOpType.add,
                    )
                    nc.sync.dma_start(
                        y_d.ap()[oc][:, tcn * 512:(tcn + 1) * 512], ot[:],
                    )
                else:
                    # split the final store across engines to trim the tail
                    nc.vector.tensor_scalar(
                        ot[:, 0:256], ps[tcn, oc][:, 0:256],
                        bias_t[:, oc:oc + 1], None, mybir.AluOpType.add,
                    )
                    nc.scalar.activation(
                        ot[:, 256:512], ps[tcn, oc][:, 256:512],
                        mybir.ActivationFunctionType.Identity,
                        bias=bias_t[:, oc:oc + 1],
                    )
                    c0 = tcn * 512
                    nc.sync.dma_start(
                        y_d.ap()[oc][:, c0:c0 + 256], ot[:, 0:256])
                    nc.scalar.dma_start(
                        y_d.ap()[oc][:, c0 + 256:c0 + 512], ot[:, 256:512])

            for ic in range(2):
                for tcn in range(n_tc):
                    for oc in range(2):
                        for td in range(Td):
                            if ic == 1 and td == Td - 1 and not S:
                                nc.tensor.matmul(
                                    ps[tcn, oc][:],
                                    kt_t[ic][:, td,
                                             oc * 128:(oc + 1) * 128],
                                    xp[ic][:, dense[td] + tcn * 512:
                                           dense[td] + tcn * 512 + 512],
                                    start=False, stop=True,
                                )
                            else:
                                mm(ic, tcn, oc, td)
                        if ic == 1:
                            close(tcn, oc)

    nc.compile()
    return nc


def kernel(x, weight, P, bias):
    global LAST_EXEC_NS, LAST_TRACE_PATH
    x = np.ascontiguousarray(np.asarray(x, dtype=np.float32))
    bias = np.asarray(bias, dtype=np.float32)

    dmin, ktaps = _host_fold_kernel(weight, P)
    T = ktaps.shape[0]
    dense, strips = _classify_taps(ktaps)
    if not dense:  # degenerate: keep at least one dense tap
        if strips:
            dense = [strips[-1][0]]
            strips = [s for s in strips if s[0] != dense[0]]
        else:
            dense = [0]  # all-zero kernel: output is just the bias
    strip_taps = tuple(t for t, _ in strips)
    S = len(strips)

    if DTYPE == "bf16":
        import ml_dtypes
        pdt_np = ml_dtypes.bfloat16
    else:
        pdt_np = np.float32

    key = (T, tuple(dense), strip_taps, DTYPE)
    if key not in _BUILD_CACHE:
        _BUILD_CACHE[key] = _build(T, dense, strip_taps, DTYPE)
    nc = _BUILD_CACHE[key]

    # kt layout (ic_chunk, i_within, dense_tap, o): per-ic rows contiguous
    kt_in = np.ascontiguousarray(
        ktaps[dense].reshape(len(dense), 2, 128, OC)
        .transpose(1, 2, 0, 3).astype(pdt_np))
    b_in = np.ascontiguousarray(bias.reshape(2, 128).T)

    # pad x on the host so the device needs no memset: column c of xpad is
    # x index c + (dmin - PAD)
    W = LEN + T - 1
    zl = max(0, PAD - dmin)
    xs = max(0, dmin - PAD)
    xn = min(LEN - xs, W - zl)
    xpad = np.zeros((N_CORES, 2, 128, W), dtype=pdt_np)
    xpad[:, :, :, zl:zl + xn] = (
        x.reshape(N_CORES, 2, 128, LEN)[:, :, :, xs:xs + xn].astype(pdt_np))

    if S:
        ktp_in = np.zeros((SPARSE_ROWS, OC), dtype=pdt_np)
        xg_in = np.zeros((N_CORES, SPARSE_ROWS, LEN), dtype=pdt_np)
        flat_x = xpad.reshape(N_CORES, 256, W)
        r0 = 0
        for t_sp, rows in strips:
            nr = len(rows)
            ktp_in[r0:r0 + nr] = ktaps[t_sp][rows].astype(pdt_np)
            # pre-shift: packed row reads xpad[row, c + t_sp]
            xg_in[:, r0:r0 + nr] = flat_x[:, rows, t_sp:t_sp + LEN]
            r0 += nr

    in_maps = []
    for c in range(N_CORES):
        m = {"x": xpad[c], "kt": kt_in, "bias": b_in}
        if S:
            m["ktp"] = ktp_in
            m["xg"] = xg_in[c]
        in_maps.append(m)

    kwargs = {}
    bass_utils.upload_artifacts = lambda tmpdir: tmpdir
    if TRACE:
        kwargs["trace"] = True
    res = None
    for attempt in range(3):
        try:
            res = bass_utils.run_bass_kernel_spmd(
                nc, in_maps, core_ids=list(range(N_CORES)), **kwargs
            )
            break
        except Exception:
            # occasional transient NRT_EXEC_UNIT_UNRECOVERABLE on this
            # fabric; give the device a moment to recover, then retry
            if attempt == 2:
                raise
            import time
            time.sleep(3.0)
    if TRACE:
        LAST_EXEC_NS = res.exec_time_ns
        if res.instructions_and_trace is not None:
            LAST_TRACE_PATH = res.instructions_and_trace[1]

    out = np.empty((N, OC, LEN), dtype=np.float32)
    for c in range(N_CORES):
        out[c] = res.results[c]["out"].reshape(OC, LEN)
    return out


# revision 31
# speedup vs baseline: 1.0406x; 1.0068x over previous
"""Dcls1d (dilated conv1d with learnable spacings) on 8 Trainium2 NeuronCores.

Problem: x (8, 256, 2048) f32; weight (256, 256, 16); P (1, 256, 256, 16);
bias (256,). A dense conv kernel (O=256, I=256, DKS=33) is built from
weight/P by linear interpolation at positions P, then conv1d(x, kern,
pad=16) + bias -> out (8, 256, 2048).

Strategy:
 - Host: fold (weight, P) -> per-tap matmul weights, keeping only taps that
   are actually nonzero (P = clip(0.5*randn, +-16) clusters around the
   center tap, so typically only ~7 of 33 taps carry weight). Edge taps are
   extremely sparse (a handful of nonzero input channels) - those become
   32-row "strip" matmuls packed into PE row-tiles instead of full 128-deep
   matmuls.
 - Device: data-parallel over batch, one batch element per NeuronCore.
   out[o, t] = sum_d kern[d][:, o] . x[:, t + d - 16], accumulated in PSUM
   over 2 input-channel chunks x dense taps (+ sparse strips), 2x4 output
   tiles of (128, 512); bias added on the PSUM->SBUF move.
"""

import numpy as np

try:
    import concourse  # noqa: F401
except ImportError:  # pragma: no cover - container fallback
    import sys

    sys.path.insert(0, "/opt/trn_rl_repo")

import concourse.bacc as bacc
import concourse.mybir as mybir
import concourse.tile as tile
import concourse.bass_utils as bass_utils

DKS = 33
PAD = 16
N, IC, LEN = 8, 256, 2048
OC = 256
KC = 16
N_CORES = 8
SPARSE_ROWS = 128  # packed contraction depth: full 128 partitions
# (64-partition operands stream at half SBUF bandwidth on the PE)

TRACE = False  # test harness sets kernel_mod.TRACE = True to profile
DTYPE = "f32r"  # "f32r" (safe, ~1.5e-4 rel err) or "bf16" (faster, ~3e-3)
LAST_EXEC_NS = None
LAST_TRACE_PATH = None

_BUILD_CACHE = {}


def _host_fold_kernel(weight, P):
    """Reproduce reference construct_kernel for the active taps only.

    Returns (dmin, ktaps) with ktaps[t, i, o] the lhsT-layout weights for
    tap d = dmin + t. Mirrors the reference arithmetic in fp32:
    kern[o,i,d] = sum_kc w[o,i,kc] * (W1 + frac*(W2-W1)).
    """
    w = np.asarray(weight, dtype=np.float32)
    Pf32 = np.asarray(P, dtype=np.float32)
    Pp = Pf32 + np.float32(DKS // 2)
    Pf = np.floor(Pp)
    frac = (Pp - Pf)[0, 0]  # (IC, KC) - out-channel 0's fractional part
    P1 = Pf[0]  # (OC, IC, KC)

    dmin = max(0, int(P1.min()))
    dmax = min(DKS - 1, int(P1.max()) + 1)
    dd = np.arange(dmin, dmax + 1, dtype=np.float32)
    W1 = dd[:, None, None, None] == P1[None]
    W2 = dd[:, None, None, None] == (P1 + 1)[None]
    K = W1.astype(np.float32) + frac[None, None] * (
        W2.astype(np.float32) - W1.astype(np.float32)
    )
    kern = (w[None] * K).sum(-1)  # (T, OC, IC)
    ktaps = np.ascontiguousarray(kern.transpose(0, 2, 1))  # (T, IC, OC)
    return dmin, ktaps


def _classify_taps(ktaps):
    """Split taps into dense ones and sparse (tap, nonzero-rows) ones.

    Sparse taps' (tap, input-row) pairs are packed -- with the row's x data
    pre-shifted by the tap offset on the host -- into one K<=SPARSE_ROWS
    matmul, so the tap budget is a row budget.
    """
    T = ktaps.shape[0]
    nzrows = [np.nonzero(np.any(ktaps[t] != 0, axis=1))[0] for t in range(T)]
    order = sorted(range(T), key=lambda t: len(nzrows[t]))
    strips = []
    sparse = set()
    used = 0
    for t in order:
        n = len(nzrows[t])
        if n == 0:
            sparse.add(t)  # all-zero tap: drop entirely
        elif n <= 48 and used + n <= SPARSE_ROWS:
            strips.append((t, nzrows[t]))
            sparse.add(t)
            used += n
    dense = [t for t in range(T) if t not in sparse]
    strips.sort()
    return dense, strips


def _build(T, dense, strip_taps, dtype_name):
    f32 = mybir.dt.float32
    pdt = mybir.dt.float32r if dtype_name == "f32r" else mybir.dt.bfloat16

    W = LEN + T - 1  # host-padded x width; tap t reads cols [t+c0, t+c0+512)
    n_tc = LEN // 512
    Td = len(dense)
    S = len(strip_taps)

    nc = bacc.Bacc("TRN2", target_bir_lowering=False, debug=False,
                   num_devices=N_CORES)
    x_d = nc.dram_tensor("x", (2, 128, W), pdt, kind="ExternalInput")
    kt_d = nc.dram_tensor("kt", (2, 128, Td, OC), pdt, kind="ExternalInput")
    b_d = nc.dram_tensor("bias", (128, 2), f32, kind="ExternalInput")
    if S:
        ktp_d = nc.dram_tensor("ktp", (SPARSE_ROWS, OC), pdt,
                               kind="ExternalInput")
        xg_d = nc.dram_tensor("xg", (SPARSE_ROWS, LEN), pdt,
                              kind="ExternalInput")
    y_d = nc.dram_tensor("out", (2, 128, LEN), f32, kind="ExternalOutput")

    with tile.TileContext(nc) as tc:
        with (
            tc.tile_pool(name="const", bufs=1) as cpool,
            tc.tile_pool(name="ps", bufs=8, space="PSUM") as pspool,
            tc.tile_pool(name="outp", bufs=4) as opool,
        ):
            kt_t = [cpool.tile([128, Td, OC], pdt, tag=f"kt{ic}",
                               name=f"kt{ic}") for ic in range(2)]
            xp = [cpool.tile([128, W], pdt, tag=f"xp{ic}",
                             name=f"xp{ic}") for ic in range(2)]
            bias_t = cpool.tile([128, 2], f32, tag="bias")
            if S:
                ktp_t = cpool.tile([SPARSE_ROWS, OC], pdt, tag="ktp",
                                   name="ktp")
                xg_t = cpool.tile([SPARSE_ROWS, LEN], pdt, tag="xg",
                                  name="xg")

            # PE warmup: the HAM clock gate holds the PE at 1.2 GHz until it
            # has been busy ~3.4us; dummy matmuls cover the DMA fill so real
            # matmuls run at 2.4 GHz.
            warm = cpool.tile([128, 512], pdt, tag="warm")
            nc.vector.memset(
                warm[:].bitcast(f32 if dtype_name == "f32r"
                                else mybir.dt.bfloat16), 0.0)
            wps = pspool.tile([64, 512], f32, tag="ps", name="warm_ps")
            for i in range(16):
                nc.tensor.matmul(wps[:], warm[:, 0:64], warm[:],
                                 start=True, stop=True)

            # DMA completion is ~serial per stream (sync / scalar each drive
            # their own HWDGE ring); order each stream by when the PE needs
            # the data. The matmul stream is phase-split: all 8 output
            # groups' ic0 halves first (needing only kt0 + xp0), then ic1.
            dh = max(1, min(3, Td))  # first kt chunk: dense taps [0, dh)
            ch = 1024 + T - 1  # xp column split: tcn0+1 read [0, ch)
            cq = min(768, ch)  # tcn0 window only needs cols [0, 512+T)
            nc.sync.dma_start(kt_t[0][:, 0:dh], kt_d.ap()[0][:, 0:dh])
            nc.scalar.dma_start(xp[0][:, 0:cq], x_d.ap()[0][:, 0:cq])
            if dh < Td:
                nc.sync.dma_start(kt_t[0][:, dh:Td], kt_d.ap()[0][:, dh:Td])
            nc.scalar.dma_start(xp[0][:, cq:ch], x_d.ap()[0][:, cq:ch])
            nc.scalar.dma_start(xp[0][:, ch:W], x_d.ap()[0][:, ch:W])
            nc.sync.dma_start(kt_t[1][:, 0:dh], kt_d.ap()[1][:, 0:dh])
            nc.scalar.dma_start(xp[1][:, 0:cq], x_d.ap()[1][:, 0:cq])
            if dh < Td:
                nc.sync.dma_start(kt_t[1][:, dh:Td], kt_d.ap()[1][:, dh:Td])
            nc.scalar.dma_start(xp[1][:, cq:W], x_d.ap()[1][:, cq:W])
            nc.sync.dma_start(bias_t[:], b_d.ap())
            if S:
                nc.sync.dma_start(ktp_t[:], ktp_d.ap())
                nc.scalar.dma_start(xg_t[:], xg_d.ap())

            ps = {}
            for tcn in range(n_tc):
                for oc in range(2):
                    ps[tcn, oc] = pspool.tile([128, 512], f32, tag="ps",
                                              name=f"ps_{tcn}_{oc}")

            def mm(ic, tcn, oc, td):
                nc.tensor.matmul(
                    ps[tcn, oc][:],
                    kt_t[ic][:, td, oc * 128:(oc + 1) * 128],
                    xp[ic][:, dense[td] + tcn * 512:
                           dense[td] + tcn * 512 + 512],
                    start=(ic == 0 and td == 0),
                    stop=False,
                )

            def strip_mms(tcn, oc):
                # all sparse taps in one K=SPARSE_ROWS matmul: each packed
                # row's x was pre-shifted by its tap offset on the host, so
                # every row reads the same column window
                c0 = tcn * 512
                nc.tensor.matmul(
                    ps[tcn, oc][:],
                    ktp_t[:, oc * 128:(oc + 1) * 128],
                    xg_t[:, c0:c0 + 512],
                    start=False, stop=True,
                )

            def close(tcn, oc):
                if S:
                    strip_mms(tcn, oc)
                else:
                    # re-emit nothing; mark group end via last dense mm's
                    # stop flag instead
                    pass
                ot = opool.tile([128, 512], f32, tag="out",
                                name=f"ot_{tcn}_{oc}")
                last = (tcn == n_tc - 1 and oc == 1)
                if not last:
                    nc.vector.tensor_scalar(
                        ot[:], ps[tcn, oc][:], bias_t[:, oc:oc + 1],
                        None, mybir.AluOpType.add,
                    )
                    nc.sync.dma_start(
                        y_d.ap()[oc][:, tcn * 512:(tcn + 1) * 512], ot[:],
                    )
                else:
                    # split the final store across engines to trim the tail
                    nc.vector.tensor_scalar(
                        ot[:, 0:256], ps[tcn, oc][:, 0:256],
                        bias_t[:, oc:oc + 1], None, mybir.AluOpType.add,
                    )
                    nc.scalar.activation(
                        ot[:, 256:512], ps[tcn, oc][:, 256:512],
                        mybir.ActivationFunctionType.Identity,
                        bias=bias_t[:, oc:oc + 1],
                    )
                    c0 = tcn * 512
                    nc.sync.dma_start(
                        y_d.ap()[oc][:, c0:c0 + 256], ot[:, 0:256])
                    nc.scalar.dma_start(
                        y_d.ap()[oc][:, c0 + 256:c0 + 512], ot[:, 256:512])

            for ic in range(2):
                for tcn in range(n_tc):
                    for oc in range(2):
                        for td in range(Td):
                            if ic == 1 and td == Td - 1 and not S:
                                nc.tensor.matmul(
                                    ps[tcn, oc][:],
                                    kt_t[ic][:, td,
                                             oc * 128:(oc + 1) * 128],
                                    xp[ic][:, dense[td] + tcn * 512:
                                           dense[td] + tcn * 512 + 512],
                                    start=False, stop=True,
                                )
                            else:
                                mm(ic, tcn, oc, td)
                        if ic == 1:
                            close(tcn, oc)

    nc.compile()
    return nc


def kernel(x, weight, P, bias):
    global LAST_EXEC_NS, LAST_TRACE_PATH
    x = np.ascontiguousarray(np.asarray(x, dtype=np.float32))
    bias = np.asarray(bias, dtype=np.float32)

    dmin, ktaps = _host_fold_kernel(weight, P)
    T = ktaps.shape[0]
    dense, strips = _classify_taps(ktaps)
    if not dense:  # degenerate: keep at least one dense tap
        if strips:
            dense = [strips[-1][0]]
            strips = [s for s in strips if s[0] != dense[0]]
        else:
            dense = [0]  # all-zero kernel: output is just the bias
    strip_taps = tuple(t for t, _ in strips)
    S = len(strips)

    if DTYPE == "bf16":
        import ml_dtypes
        pdt_np = ml_dtypes.bfloat16
    else:
        pdt_np = np.float32

    key = (T, tuple(dense), strip_taps, DTYPE)
    if key not in _BUILD_CACHE:
        _BUILD_CACHE[key] = _build(T, dense, strip_taps, DTYPE)
    nc = _BUILD_CACHE[key]

    # kt layout (ic_chunk, i_within, dense_tap, o): per-ic rows contiguous
    kt_in = np.ascontiguousarray(
        ktaps[dense].reshape(len(dense), 2, 128, OC)
        .transpose(1, 2, 0, 3).astype(pdt_np))
    b_in = np.ascontiguousarray(bias.reshape(2, 128).T)

    # pad x on the host so the device needs no memset: column c of xpad is
    # x index c + (dmin - PAD)
    W = LEN + T - 1
    zl = max(0, PAD - dmin)
    xs = max(0, dmin - PAD)
    xn = min(LEN - xs, W - zl)
    xpad = np.zeros((N_CORES, 2, 128, W), dtype=pdt_np)
    xpad[:, :, :, zl:zl + xn] = (
        x.reshape(N_CORES, 2, 128, LEN)[:, :, :, xs:xs + xn].astype(pdt_np))

    if S:
        ktp_in = np.zeros((SPARSE_ROWS, OC), dtype=pdt_np)
        xg_in = np.zeros((N_CORES, SPARSE_ROWS, LEN), dtype=pdt_np)
        flat_x = xpad.reshape(N_CORES, 256, W)
        r0 = 0
        for t_sp, rows in strips:
            nr = len(rows)
            ktp_in[r0:r0 + nr] = ktaps[t_sp][rows].astype(pdt_np)
            # pre-shift: packed row reads xpad[row, c + t_sp]
            xg_in[:, r0:r0 + nr] = flat_x[:, rows, t_sp:t_sp + LEN]
            r0 += nr

    in_maps = []
    for c in range(N_CORES):
        m = {"x": xpad[c], "kt": kt_in, "bias": b_in}
        if S:
            m["ktp"] = ktp_in
            m["xg"] = xg_in[c]
        in_maps.append(m)

    kwargs = {}
    bass_utils.upload_artifacts = lambda tmpdir: tmpdir
    if TRACE:
        kwargs["trace"] = True
    res = None
    for attempt in range(3):
        try:
            res = bass_utils.run_bass_kernel_spmd(
                nc, in_maps, core_ids=list(range(N_CORES)), **kwargs
            )
            break
        except Exception:
            # occasional transient NRT_EXEC_UNIT_UNRECOVERABLE on this
            # fabric; give the device a moment to recover, then retry
            if attempt == 2:
                raise
            import time
            time.sleep(3.0)
    if TRACE:
        LAST_EXEC_NS = res.exec_time_ns
        if res.instructions_and_trace is not None:
            LAST_TRACE_PATH = res.instructions_and_trace[1]

    out = np.empty((N, OC, LEN), dtype=np.float32)
    for c in range(N_CORES):
        out[c] = res.results[c]["out"].reshape(OC, LEN)
    return out


# revision 32
# speedup vs baseline: 1.0572x; 1.0160x over previous
"""Dcls1d (dilated conv1d with learnable spacings) on 8 Trainium2 NeuronCores.

Problem: x (8, 256, 2048) f32; weight (256, 256, 16); P (1, 256, 256, 16);
bias (256,). A dense conv kernel (O=256, I=256, DKS=33) is built from
weight/P by linear interpolation at positions P, then conv1d(x, kern,
pad=16) + bias -> out (8, 256, 2048).

Strategy:
 - Host: fold (weight, P) -> per-tap matmul weights, keeping only taps that
   are actually nonzero (P = clip(0.5*randn, +-16) clusters around the
   center tap, so typically only ~7 of 33 taps carry weight). Edge taps are
   extremely sparse (a handful of nonzero input channels) - those become
   32-row "strip" matmuls packed into PE row-tiles instead of full 128-deep
   matmuls.
 - Device: data-parallel over batch, one batch element per NeuronCore.
   out[o, t] = sum_d kern[d][:, o] . x[:, t + d - 16], accumulated in PSUM
   over 2 input-channel chunks x dense taps (+ sparse strips), 2x4 output
   tiles of (128, 512); bias added on the PSUM->SBUF move.
"""

import numpy as np

try:
    import concourse  # noqa: F401
except ImportError:  # pragma: no cover - container fallback
    import sys

    sys.path.insert(0, "/opt/trn_rl_repo")

import concourse.bacc as bacc
import concourse.mybir as mybir
import concourse.tile as tile
import concourse.bass_utils as bass_utils

DKS = 33
PAD = 16
N, IC, LEN = 8, 256, 2048
OC = 256
KC = 16
N_CORES = 8
SPARSE_ROWS = 128  # packed contraction depth: full 128 partitions
# (64-partition operands stream at half SBUF bandwidth on the PE)

TRACE = False  # test harness sets kernel_mod.TRACE = True to profile
DTYPE = "f32r"  # "f32r" (safe, ~1.5e-4 rel err) or "bf16" (faster, ~3e-3)
LAST_EXEC_NS = None
LAST_TRACE_PATH = None

_BUILD_CACHE = {}


def _host_fold_kernel(weight, P):
    """Reproduce reference construct_kernel for the active taps only.

    Returns (dmin, ktaps) with ktaps[t, i, o] the lhsT-layout weights for
    tap d = dmin + t. Mirrors the reference arithmetic in fp32:
    kern[o,i,d] = sum_kc w[o,i,kc] * (W1 + frac*(W2-W1)).
    """
    w = np.asarray(weight, dtype=np.float32)
    Pf32 = np.asarray(P, dtype=np.float32)
    Pp = Pf32 + np.float32(DKS // 2)
    Pf = np.floor(Pp)
    frac = (Pp - Pf)[0, 0]  # (IC, KC) - out-channel 0's fractional part
    P1 = Pf[0]  # (OC, IC, KC)

    dmin = max(0, int(P1.min()))
    dmax = min(DKS - 1, int(P1.max()) + 1)
    dd = np.arange(dmin, dmax + 1, dtype=np.float32)
    W1 = dd[:, None, None, None] == P1[None]
    W2 = dd[:, None, None, None] == (P1 + 1)[None]
    K = W1.astype(np.float32) + frac[None, None] * (
        W2.astype(np.float32) - W1.astype(np.float32)
    )
    kern = (w[None] * K).sum(-1)  # (T, OC, IC)
    ktaps = np.ascontiguousarray(kern.transpose(0, 2, 1))  # (T, IC, OC)
    return dmin, ktaps


def _classify_taps(ktaps):
    """Split taps into dense ones and sparse (tap, nonzero-rows) ones.

    Sparse taps' (tap, input-row) pairs are packed -- with the row's x data
    pre-shifted by the tap offset on the host -- into one K<=SPARSE_ROWS
    matmul, so the tap budget is a row budget.
    """
    T = ktaps.shape[0]
    nzrows = [np.nonzero(np.any(ktaps[t] != 0, axis=1))[0] for t in range(T)]
    order = sorted(range(T), key=lambda t: len(nzrows[t]))
    strips = []
    sparse = set()
    used = 0
    for t in order:
        n = len(nzrows[t])
        if n == 0:
            sparse.add(t)  # all-zero tap: drop entirely
        elif n <= 48 and used + n <= SPARSE_ROWS:
            strips.append((t, nzrows[t]))
            sparse.add(t)
            used += n
    dense = [t for t in range(T) if t not in sparse]
    strips.sort()
    return dense, strips


def _build(T, dense, strip_taps, dtype_name):
    f32 = mybir.dt.float32
    pdt = mybir.dt.float32r if dtype_name == "f32r" else mybir.dt.bfloat16

    W = LEN + T - 1  # host-padded x width; tap t reads cols [t+c0, t+c0+512)
    n_tc = LEN // 512
    Td = len(dense)
    S = len(strip_taps)

    nc = bacc.Bacc("TRN2", target_bir_lowering=False, debug=False,
                   num_devices=N_CORES)
    x_d = nc.dram_tensor("x", (2, 128, W), pdt, kind="ExternalInput")
    kt_d = nc.dram_tensor("kt", (2, 128, Td, OC), pdt, kind="ExternalInput")
    b_d = nc.dram_tensor("bias", (128, 2), f32, kind="ExternalInput")
    if S:
        ktp_d = nc.dram_tensor("ktp", (SPARSE_ROWS, OC), pdt,
                               kind="ExternalInput")
        xg_d = nc.dram_tensor("xg", (SPARSE_ROWS, LEN), pdt,
                              kind="ExternalInput")
    y_d = nc.dram_tensor("out", (2, 128, LEN), f32, kind="ExternalOutput")

    with tile.TileContext(nc) as tc:
        with (
            tc.tile_pool(name="const", bufs=1) as cpool,
            tc.tile_pool(name="ps", bufs=8, space="PSUM") as pspool,
            tc.tile_pool(name="outp", bufs=4) as opool,
        ):
            kt_t = [cpool.tile([128, Td, OC], pdt, tag=f"kt{ic}",
                               name=f"kt{ic}") for ic in range(2)]
            xp = [cpool.tile([128, W], pdt, tag=f"xp{ic}",
                             name=f"xp{ic}") for ic in range(2)]
            bias_t = cpool.tile([128, 2], f32, tag="bias")
            if S:
                ktp_t = cpool.tile([SPARSE_ROWS, OC], pdt, tag="ktp",
                                   name="ktp")
                xg_t = cpool.tile([SPARSE_ROWS, LEN], pdt, tag="xg",
                                  name="xg")

            # PE warmup: the HAM clock gate holds the PE at 1.2 GHz until it
            # has been busy ~3.4us; dummy matmuls cover the DMA fill so real
            # matmuls run at 2.4 GHz.
            warm = cpool.tile([128, 512], pdt, tag="warm")
            nc.vector.memset(
                warm[:].bitcast(f32 if dtype_name == "f32r"
                                else mybir.dt.bfloat16), 0.0)
            wps = pspool.tile([64, 512], f32, tag="ps", name="warm_ps")
            for i in range(10):
                nc.tensor.matmul(wps[:], warm[:, 0:64], warm[:],
                                 start=True, stop=True)

            # DMA completion is ~serial per stream (sync / scalar each drive
            # their own HWDGE ring); order each stream by when the PE needs
            # the data. The matmul stream is phase-split: all 8 output
            # groups' ic0 halves first (needing only kt0 + xp0), then ic1.
            dh = max(1, min(3, Td))  # first kt chunk: dense taps [0, dh)
            ch = 1024 + T - 1  # xp column split: tcn0+1 read [0, ch)
            cq = min(768, ch)  # tcn0 window only needs cols [0, 512+T)
            nc.sync.dma_start(kt_t[0][:, 0:dh], kt_d.ap()[0][:, 0:dh])
            nc.scalar.dma_start(xp[0][:, 0:cq], x_d.ap()[0][:, 0:cq])
            if dh < Td:
                nc.sync.dma_start(kt_t[0][:, dh:Td], kt_d.ap()[0][:, dh:Td])
            nc.scalar.dma_start(xp[0][:, cq:ch], x_d.ap()[0][:, cq:ch])
            nc.scalar.dma_start(xp[0][:, ch:W], x_d.ap()[0][:, ch:W])
            nc.sync.dma_start(kt_t[1][:, 0:dh], kt_d.ap()[1][:, 0:dh])
            nc.scalar.dma_start(xp[1][:, 0:cq], x_d.ap()[1][:, 0:cq])
            if dh < Td:
                nc.sync.dma_start(kt_t[1][:, dh:Td], kt_d.ap()[1][:, dh:Td])
            nc.scalar.dma_start(xp[1][:, cq:W], x_d.ap()[1][:, cq:W])
            nc.sync.dma_start(bias_t[:], b_d.ap())
            if S:
                nc.sync.dma_start(ktp_t[:], ktp_d.ap())
                nc.scalar.dma_start(xg_t[:], xg_d.ap())

            ps = {}
            for tcn in range(n_tc):
                for oc in range(2):
                    ps[tcn, oc] = pspool.tile([128, 512], f32, tag="ps",
                                              name=f"ps_{tcn}_{oc}")

            def mm(ic, tcn, oc, td):
                nc.tensor.matmul(
                    ps[tcn, oc][:],
                    kt_t[ic][:, td, oc * 128:(oc + 1) * 128],
                    xp[ic][:, dense[td] + tcn * 512:
                           dense[td] + tcn * 512 + 512],
                    start=(ic == 0 and td == 0),
                    stop=False,
                )

            def strip_mms(tcn, oc):
                # all sparse taps in one K=SPARSE_ROWS matmul: each packed
                # row's x was pre-shifted by its tap offset on the host, so
                # every row reads the same column window
                c0 = tcn * 512
                nc.tensor.matmul(
                    ps[tcn, oc][:],
                    ktp_t[:, oc * 128:(oc + 1) * 128],
                    xg_t[:, c0:c0 + 512],
                    start=False, stop=True,
                )

            def close(tcn, oc):
                if S:
                    strip_mms(tcn, oc)
                else:
                    # re-emit nothing; mark group end via last dense mm's
                    # stop flag instead
                    pass
                ot = opool.tile([128, 512], f32, tag="out",
                                name=f"ot_{tcn}_{oc}")
                last = (tcn == n_tc - 1 and oc == 1)
                if not last:
                    nc.vector.tensor_scalar(
                        ot[:], ps[tcn, oc][:], bias_t[:, oc:oc + 1],
                        None, mybir.AluOpType.add,
                    )
                    nc.sync.dma_start(
                        y_d.ap()[oc][:, tcn * 512:(tcn + 1) * 512], ot[:],
                    )
                else:
                    # split the final store across engines to trim the tail
                    nc.vector.tensor_scalar(
                        ot[:, 0:256], ps[tcn, oc][:, 0:256],
                        bias_t[:, oc:oc + 1], None, mybir.AluOpType.add,
                    )
                    nc.scalar.activation(
                        ot[:, 256:512], ps[tcn, oc][:, 256:512],
                        mybir.ActivationFunctionType.Identity,
                        bias=bias_t[:, oc:oc + 1],
                    )
                    c0 = tcn * 512
                    nc.sync.dma_start(
                        y_d.ap()[oc][:, c0:c0 + 256], ot[:, 0:256])
                    nc.scalar.dma_start(
                        y_d.ap()[oc][:, c0 + 256:c0 + 512], ot[:, 256:512])

            for ic in range(2):
                for tcn in range(n_tc):
                    for oc in range(2):
                        for td in range(Td):
                            if ic == 1 and td == Td - 1 and not S:
                                nc.tensor.matmul(
                                    ps[tcn, oc][:],
                                    kt_t[ic][:, td,
                                             oc * 128:(oc + 1) * 128],
                                    xp[ic][:, dense[td] + tcn * 512:
                                           dense[td] + tcn * 512 + 512],
                                    start=False, stop=True,
                                )
                            else:
                                mm(ic, tcn, oc, td)
                        if ic == 1:
                            close(tcn, oc)

    nc.compile()
    return nc


def kernel(x, weight, P, bias):
    global LAST_EXEC_NS, LAST_TRACE_PATH
    x = np.ascontiguousarray(np.asarray(x, dtype=np.float32))
    bias = np.asarray(bias, dtype=np.float32)

    dmin, ktaps = _host_fold_kernel(weight, P)
    T = ktaps.shape[0]
    dense, strips = _classify_taps(ktaps)
    if not dense:  # degenerate: keep at least one dense tap
        if strips:
            dense = [strips[-1][0]]
            strips = [s for s in strips if s[0] != dense[0]]
        else:
            dense = [0]  # all-zero kernel: output is just the bias
    strip_taps = tuple(t for t, _ in strips)
    S = len(strips)

    if DTYPE == "bf16":
        import ml_dtypes
        pdt_np = ml_dtypes.bfloat16
    else:
        pdt_np = np.float32

    key = (T, tuple(dense), strip_taps, DTYPE)
    if key not in _BUILD_CACHE:
        _BUILD_CACHE[key] = _build(T, dense, strip_taps, DTYPE)
    nc = _BUILD_CACHE[key]

    # kt layout (ic_chunk, i_within, dense_tap, o): per-ic rows contiguous
    kt_in = np.ascontiguousarray(
        ktaps[dense].reshape(len(dense), 2, 128, OC)
        .transpose(1, 2, 0, 3).astype(pdt_np))
    b_in = np.ascontiguousarray(bias.reshape(2, 128).T)

    # pad x on the host so the device needs no memset: column c of xpad is
    # x index c + (dmin - PAD)
    W = LEN + T - 1
    zl = max(0, PAD - dmin)
    xs = max(0, dmin - PAD)
    xn = min(LEN - xs, W - zl)
    xpad = np.zeros((N_CORES, 2, 128, W), dtype=pdt_np)
    xpad[:, :, :, zl:zl + xn] = (
        x.reshape(N_CORES, 2, 128, LEN)[:, :, :, xs:xs + xn].astype(pdt_np))

    if S:
        ktp_in = np.zeros((SPARSE_ROWS, OC), dtype=pdt_np)
        xg_in = np.zeros((N_CORES, SPARSE_ROWS, LEN), dtype=pdt_np)
        flat_x = xpad.reshape(N_CORES, 256, W)
        r0 = 0
        for t_sp, rows in strips:
            nr = len(rows)
            ktp_in[r0:r0 + nr] = ktaps[t_sp][rows].astype(pdt_np)
            # pre-shift: packed row reads xpad[row, c + t_sp]
            xg_in[:, r0:r0 + nr] = flat_x[:, rows, t_sp:t_sp + LEN]
            r0 += nr

    in_maps = []
    for c in range(N_CORES):
        m = {"x": xpad[c], "kt": kt_in, "bias": b_in}
        if S:
            m["ktp"] = ktp_in
            m["xg"] = xg_in[c]
        in_maps.append(m)

    kwargs = {}
    bass_utils.upload_artifacts = lambda tmpdir: tmpdir
    if TRACE:
        kwargs["trace"] = True
    res = None
    for attempt in range(3):
        try:
            res = bass_utils.run_bass_kernel_spmd(
                nc, in_maps, core_ids=list(range(N_CORES)), **kwargs
            )
            break
        except Exception:
            # occasional transient NRT_EXEC_UNIT_UNRECOVERABLE on this
            # fabric; give the device a moment to recover, then retry
            if attempt == 2:
                raise
            import time
            time.sleep(3.0)
    if TRACE:
        LAST_EXEC_NS = res.exec_time_ns
        if res.instructions_and_trace is not None:
            LAST_TRACE_PATH = res.instructions_and_trace[1]

    out = np.empty((N, OC, LEN), dtype=np.float32)
    for c in range(N_CORES):
        out[c] = res.results[c]["out"].reshape(OC, LEN)
    return out
